# revision 39
# baseline (speedup 1.0000x reference)
"""OCS fused kernel for Trainium2, data-parallel over batch across 8 cores.

Algebraic restructuring (verified vs reference to ~1e-6 in fp64):

Spatial branch (4 scan orders, shared weights) collapses to a symmetric
5-point stencil with scan-order wrap rules, and the two 1x1 convs fold
through it:  W_proj @ y_sp = A2 @ (4-neighbor sum of x) + (B3 - W_proj) @ x.
The 4-neighbor sums are not materialized: A2 is applied as two fp8
DoubleRow matmuls, each fusing a +-shift pair of x (second K-half read via
a strided AP view), with the A2 magnitude rescaled 2^4 into fp8 range and
x cast to fp8 * 2^-4 on-device by the ACT pass that also accumulates the
channel-gate row sums (A2 term is ~0.7% of y, fp8 error is negligible
there). Col-scan wraps are two extra small bf16 matmuls.
Channel branch: m = g g^T is rank-1, so the whole conv pipeline collapses
into three [32,128] matmuls on shifted x (weights MP/MQ/MR = u (x) P/Q/R
built on-device from g = sum of x), a silu, and one [128,32] matmul.
Diff branch (large contributor, so it stays bf16): |dx| per axis once,
pair-sums as shifted adds on DVE, W_proj folds in as W_d @ H + W_d @ V.
BatchNorm: per-core partial (sum, sumsq) -> 1KB AllReduce (warmed up by a
dummy collective at kernel start so the mesh-algo load is off the critical
path) -> affine applied at DVE 2x with bf16 output (host upcasts to f32).
"""

import numpy as np
import ml_dtypes

B, C, Himg, Wimg = 8, 128, 128, 128
L = Himg * Wimg            # 16384
NCORES = 8
NCH = 512                  # matmul chunk columns
NCHUNK = L // NCH          # 32
NPAIR = NCHUNK // 2        # chunk pairs -> 2-bank psum tiles
NW = 2048                  # elementwise window columns (4 chunks)
NGRP = L // NW             # 8
NROW = NW // Wimg          # image rows per window (16)
EPS_BN = 1e-5
NTOT = float(B * L)        # batchnorm population per channel
XSC = 2.0 ** -4            # host prescale of the fp8 x copy
ASC = 2.0 ** 4             # fp8 A2 weight upscale (cancels XSC)

_CACHE = {}


def _make_patched_tc():
    """TileContext whose exit drain splits sem waits one-per-Drain.

    The walrus build in this container rejects Drain instructions carrying
    more than one sem wait ("Too many sync wait commands"). Stock
    TileContext attaches the whole global vector clock to a single tail
    Drain; emit one Drain per outstanding proc instead.
    """
    import bass_rust
    import concourse.tile as tile
    from concourse.vector_clock import ScopedClock

    class PatchedTC(tile.TileContext):
        def _drain_and_barrier(self, tick_clock, wait_clock):
            gc = list(tick_clock.global_clock)
            for i, v in enumerate(gc):
                if v:
                    single = [0] * len(gc)
                    single[i] = v
                    d = self.nc.sync.drain()
                    wait_clock.add_sem_waits(
                        d.ins, ScopedClock({None: bass_rust.VectorClock(single)})
                    )
            self.nc.all_engine_barrier()
            assert self.sems is not None
            popped = self.nc._tile_sem_poison_stack.pop()
            assert popped is self._sem_poison
            self.nc.clear_and_free_semaphores(list(self.sems.allocated().values()))
            self.nc.all_engine_barrier()

    return PatchedTC


def _split_excess_waits(nc):
    """Walrus here allows one sem wait per instruction; hoist extras onto
    same-engine NoOps inserted immediately before the instruction."""
    import bass_rust

    nid = 0
    for blk in nc.main_func.blocks:
        out = []
        for ins in blk.instructions:
            si = getattr(ins, "sync_info", None)
            waits = list(si.on_wait) if si is not None else []
            if len(waits) > 1:
                for w in waits[:-1]:
                    nid += 1
                    nop = bass_rust.InstNoOp(
                        name=f"I-waitsplit-{nid}", ins=[], outs=[])
                    nop.engine = ins.engine
                    nop.sync_info = bass_rust.SyncInfo(
                        on_wait=[w], on_update=[])
                    nc.register_instruction(nop, overwrite=True)
                    out.append(nop)
                si.on_wait = [waits[-1]]
                ins.sync_info = si
            out.append(ins)
        blk.instructions = out


def _build_program():
    import concourse.bass as bass
    import concourse.mybir as mybir

    PatchedTC = _make_patched_tc()

    f32 = mybir.dt.float32
    bf16 = mybir.dt.bfloat16
    fp8 = mybir.dt.float8e4
    u16 = mybir.dt.uint16
    Alu = mybir.AluOpType
    Act = mybir.ActivationFunctionType
    X = mybir.AxisListType.X
    DR = mybir.MatmulPerfMode.DoubleRow

    nc = bass.Bass(target_bir_lowering=False, num_devices=NCORES)

    x_ext = nc.declare_dram_parameter("x", [C, L], bf16, isOutput=False)
    # packed weights: one DMA per dtype class
    wbf_ext = nc.declare_dram_parameter("wbf", [C, 4 * C], bf16,
                                        isOutput=False)
    a2d_ext = nc.declare_dram_parameter("a2d", [C, 2 * C], fp8, isOutput=False)
    wsm_ext = nc.declare_dram_parameter("wsm", [C, 7], f32, isOutput=False)
    wf32_ext = nc.declare_dram_parameter("wf32", [C, 2 * C + 35], f32,
                                         isOutput=False)
    y_ext = nc.declare_dram_parameter("y", [C, L], bf16, isOutput=True)

    with PatchedTC(nc) as tc:
        with (
            tc.tile_pool(name="wp", bufs=1) as wp,
            tc.tile_pool(name="big", bufs=1) as big,
            tc.tile_pool(name="win", bufs=3) as win,
            tc.tile_pool(name="sm", bufs=1) as sm,
            tc.tile_pool(name="dump", bufs=2) as dump,
            tc.tile_pool(name="ow", bufs=8) as owp,
            tc.tile_pool(name="yps", bufs=3, space="PSUM") as yps,
            tc.tile_pool(name="hps", bufs=1, space="PSUM") as hps,
            tc.tile_pool(name="sps", bufs=1, space="PSUM") as sps,
            tc.tile_pool(name="dram", bufs=1, space="DRAM") as dram,
        ):
            # ---- big SBUF arrays ----
            xbf = big.tile([C, L], bf16)     # x (bf16, cast on host)
            x8 = big.tile([C, L], fp8)       # x * 2^-4 (fp8, on-device cast)
            ypre = big.tile([C, L], bf16)    # pre-BN output
            h1sb = big.tile([C, NGRP * NCH], bf16)  # silu(h1) packed 4ch/grp

            gsums = sm.tile([C, NGRP], f32)
            ysum = sm.tile([C, NPAIR + 1], f32)
            ysq = sm.tile([C, NPAIR + 1], f32)

            # ---- weights + x to SBUF ----
            wbf = wp.tile([C, 4 * C], bf16)
            a2d = wp.tile([C, 2 * C], fp8)
            wsm = wp.tile([C, 7], f32)
            wf32 = wp.tile([C, 2 * C + 35], f32)
            wb3t = wbf[:, 0:C]
            wa2t = wbf[:, C:2 * C]
            wdt = wbf[:, 2 * C:3 * C]
            c2t4 = wbf[:, 3 * C:4 * C]
            wcho = wf32[:, 0:C]
            wchi = wf32[:, C:2 * C]
            wm1t = wf32[:, 2 * C:2 * C + 32]
            taps = wf32[:, 2 * C + 32:2 * C + 35]
            b1t = wsm[:, 0:1]
            bout = wsm[:, 1:2]
            gb = wsm[:, 2:4]
            bout2 = wsm[:, 4:5]
            blsq = wsm[:, 5:6]
            ones_row = wp.tile([1, C], f32)
            nc.vector.memset(ones_row, 1.0)
            # weight/x DMAs all on the gpsimd queue: it issues from ~0.5us
            # while sync/scalar queues spend ~8us in preamble
            for t, e in [(wbf, wbf_ext), (a2d, a2d_ext), (wsm, wsm_ext)]:
                nc.gpsimd.dma_start(out=t, in_=e[:])
            for g in [0, 7, 2, 3, 1, 6, 4, 5]:
                lo, hi = g * NW, (g + 1) * NW
                nc.gpsimd.dma_start(out=xbf[:, lo:hi], in_=x_ext[:, lo:hi])
            nc.gpsimd.dma_start(out=wf32, in_=wf32_ext[:])

            # warmup collective: loads the CC mesh algo while compute runs,
            # so the real stats AllReduce skips the ~11us startup
            ccw_in = dram.tile([C, 2], f32)
            ccw_out = dram.tile([C, 2], f32)
            nc.gpsimd.dma_start(out=ccw_in[:], in_=gb)
            nc.gpsimd.collective_compute(
                "AllReduce", Alu.add,
                replica_groups=[list(range(NCORES))],
                ins=[ccw_in.opt()], outs=[ccw_out.opt()])
            # row sums (scaled 2^-4, which the rank-1 channel-gate algebra
            # cancels): 6 windows ride the fp8-cast ACT pass, 2 reduce on
            # DVE — separate partial tiles so the engines run in parallel
            # (slices of one tile would serialize in the dep tracker)
            gsumsD = sm.tile([C, 2], f32)
            for i, g in enumerate([2, 3]):
                lo, hi = g * NW, (g + 1) * NW
                nc.vector.tensor_reduce(gsumsD[:, i:i + 1], xbf[:, lo:hi],
                                        X, Alu.add)
            for i, g in enumerate([0, 7, 1, 4, 5, 6]):
                lo, hi = g * NW, (g + 1) * NW
                nc.scalar.activation(x8[:, lo:hi], xbf[:, lo:hi], Act.Copy,
                                     scale=XSC,
                                     accum_out=gsums[:, i:i + 1])

            def window_arrays(g):
                """Window arrays: H, V of the diff branch."""
                G0 = g * NW
                dh = win.tile([C, NW + 1], bf16, tag="dh")
                H = win.tile([C, NW], bf16, tag="H")
                dv = win.tile([C, NW + 128], bf16, tag="dv")
                V = win.tile([C, NW], bf16, tag="V")

                # dh[j] = x[G0+j] - x[G0+j-1], j in [a, e); |.| in place
                a = 1 if g == 0 else 0
                e = NW if g == NGRP - 1 else NW + 1
                nc.vector.tensor_tensor(dh[:, a:e],
                                        xbf[:, G0 + a:G0 + e],
                                        xbf[:, G0 + a - 1:G0 + e - 1],
                                        Alu.subtract)
                dh2 = dh[:, 0:NW].rearrange("p (r c) -> p r c", c=Wimg)
                nc.vector.memset(dh2[:, :, 0:1], 0.0)   # no cross-row diffs
                nc.vector.memset(dh[:, NW:NW + 1], 0.0)
                dhu = dh.bitcast(u16)
                nc.vector.tensor_scalar(dhu[:, a:e], dhu[:, a:e], 0x7FFF,
                                        None, Alu.bitwise_and)
                # H[j] = |dh[j]| + |dh[j+1]|
                nc.vector.tensor_tensor(H, dh[:, 0:NW], dh[:, 1:NW + 1],
                                        Alu.add)
                # edge fix: col0 += |dh[row,1]| ; col127 += |dh[row,127]|
                H2 = H.rearrange("p (r c) -> p r c", c=Wimg)
                nc.vector.tensor_tensor(H2[:, :, 0:1], H2[:, :, 0:1],
                                        dh2[:, :, 1:2], Alu.add)
                nc.vector.tensor_tensor(H2[:, :, Wimg - 1:Wimg],
                                        H2[:, :, Wimg - 1:Wimg],
                                        dh2[:, :, Wimg - 1:Wimg], Alu.add)

                # dv[j] = x[G0+j] - x[G0+j-128]; |.| in place
                av = 128 if g == 0 else 0
                ev = NW if g == NGRP - 1 else NW + 128
                nc.vector.tensor_tensor(dv[:, av:ev], xbf[:, G0 + av:G0 + ev],
                                        xbf[:, G0 + av - 128:G0 + ev - 128],
                                        Alu.subtract)
                dvu = dv.bitcast(u16)
                nc.vector.tensor_scalar(dvu[:, av:ev], dvu[:, av:ev], 0x7FFF,
                                        None, Alu.bitwise_and)
                if g == 0:
                    nc.vector.memset(dv[:, 0:128], 0.0)   # row 0: no up-diff
                if g == NGRP - 1:
                    # last row reflect: pair partner := own value -> 2|dv|
                    nc.vector.tensor_copy(dv[:, NW:NW + 128],
                                          dv[:, NW - 128:NW])
                # V[j] = |dv[j]| + |dv[j+128]|
                nc.vector.tensor_tensor(V, dv[:, 0:NW], dv[:, 128:NW + 128],
                                        Alu.add)
                if g == 0:
                    # row 0 reflect: V = 2*|dv[j+128]|
                    nc.vector.tensor_tensor(V[:, 0:128], V[:, 0:128],
                                            dv[:, 128:256], Alu.add)
                return H, V

            # window 0 arrays ahead of the channel chain: the PE's first
            # chunk pair needs them at about the same time as the h1 gate
            win_arrays0 = window_arrays(0)

            # ---- channel-branch small chain (needs all of x) ----
            gsum = sm.tile([C, 1], f32)
            gsd = sm.tile([C, 1], f32)
            nc.vector.tensor_reduce(gsum, gsums[:, 0:6], X, Alu.add)
            nc.vector.tensor_reduce(gsd, gsumsD, X, Alu.add)
            # fold the DVE partials in at the ACT partials' 2^-4 scale
            nc.vector.scalar_tensor_tensor(gsum, gsd, XSC, gsum,
                                           Alu.mult, Alu.add)
            ss_ps = sps.tile([1, 1], f32, tag="sp")
            nc.tensor.matmul(ss_ps, gsum, gsum, start=True, stop=True)
            ss = sm.tile([1, 1], f32)
            nc.vector.tensor_copy(ss, ss_ps)
            rn2 = sm.tile([1, 1], f32)
            nc.vector.reciprocal(rn2, ss)          # 1 / ||gsum||^2

            v_ps = sps.tile([C, 1], f32, tag="sp")
            nc.tensor.matmul(v_ps, wcho, gsum, start=True, stop=True)
            v_sb = sm.tile([C, 1], f32)
            nc.vector.tensor_copy(v_sb, v_ps)
            pqr = sm.tile([C, 3], f32)
            for j in range(3):
                nc.vector.tensor_tensor(pqr[:, j:j + 1], v_sb, taps[:, j:j + 1],
                                        Alu.mult)
            pqr2_ps = sps.tile([C, 3], f32, tag="sp")
            nc.tensor.matmul(pqr2_ps, wchi, pqr, start=True, stop=True)
            pqr2 = sm.tile([C, 3], f32)
            nc.vector.tensor_copy(pqr2, pqr2_ps)

            u_ps = sps.tile([1, 32], f32, tag="sp")
            nc.tensor.matmul(u_ps, gsum, wm1t, start=True, stop=True)
            u_sb = sm.tile([1, 32], f32)
            nc.vector.tensor_copy(u_sb, u_ps)
            u_sc = sm.tile([1, 32], f32)
            nc.vector.tensor_scalar(u_sc, u_sb, rn2[0:1, 0:1], None, Alu.mult)
            # broadcast [1,32] -> [C,32] with a K=1 ones matmul (no DRAM trip)
            ub_ps = sps.tile([C, 32], f32, tag="sp")
            nc.tensor.matmul(ub_ps, ones_row, u_sc, start=True, stop=True)
            u_bc = sm.tile([C, 32], f32)
            nc.vector.tensor_copy(u_bc, ub_ps)

            mqt = sm.tile([C, 32], bf16)
            mpt = sm.tile([C, 32], bf16)
            mrt = sm.tile([C, 32], bf16)
            for t, j in [(mpt, 0), (mqt, 1), (mrt, 2)]:
                nc.vector.tensor_scalar(t, u_bc, pqr2[:, j:j + 1], None,
                                        Alu.mult)

            # deferred fp8-cast windows (first needed by chunk pair 4)
            for g in [2, 3]:
                lo, hi = g * NW, (g + 1) * NW
                nc.scalar.activation(x8[:, lo:hi], xbf[:, lo:hi], Act.Copy,
                                     scale=XSC)

            # ---- streaming main loop ----
            def h1_group(k):
                h1ps = hps.tile([C, NCH], f32)
                for wgt, shift in [(mqt, 0), (mpt, -1), (mrt, +1)]:
                    for j in range(4):
                        n = 4 * k + j
                        n0 = n * NCH
                        lo = n0 + shift
                        hi = n0 + NCH + shift
                        plo, phi = 0, NCH
                        if lo < 0:
                            plo, lo = 1, 0
                        if hi > L:
                            phi, hi = NCH - 1, L
                        nc.tensor.matmul(
                            h1ps[32 * j:32 * j + 32, plo:phi],
                            wgt[:, 0:32], xbf[:, lo:hi],
                            start=(shift == 0), stop=(shift == 1),
                            tile_position=(0, 32 * j))
                nc.scalar.activation(h1sb[:, k * NCH:(k + 1) * NCH], h1ps,
                                     Act.Silu, bias=b1t[:, 0:1])

            def dr_pair(base, istride, n):
                """fp8 ifmap AP reading, for each of n cols j, the K-half pair
                (x8[base+j], x8[base+istride+j]) for a DoubleRow matmul."""
                anchor = x8[:, base:base + 1]
                o = anchor.opt()
                return bass.AP(tensor=anchor.tensor, offset=o.offset,
                               ap=[list(o.ap[0]), [istride, 2], [1, n]])

            a2w = a2d.rearrange("p (i m) -> p i m", i=2)
            a2s = a2d[:, 0:C]   # single (non-DR) fp8 A2 view

            def chunk_pair(m, H, V):
                """Chunks 2m, 2m+1 into one 2-bank psum tile; the bf16
                matmuls (B3, Wd@H, Wd@V) run 1024 wide."""
                ps = yps.tile([C, 2 * NCH], f32)
                NP = 2 * NCH
                p0 = 2 * m * NCH          # first flat column of the pair
                off = (2 * m % 4) * NCH   # H/V window-local offset
                for h in range(2):
                    nc.tensor.matmul(ps[:, h * NCH:(h + 1) * NCH], wb3t,
                                     xbf[:, p0 + h * NCH:p0 + (h + 1) * NCH],
                                     start=True, stop=False)
                for h in range(2):
                    n = 2 * m + h
                    n0 = n * NCH
                    q = h * NCH
                    # A2 @ (x[l-1]+x[l+1]) as one fp8 DoubleRow matmul
                    plo = 1 if n == 0 else 0
                    phi = NCH - 1 if n == NCHUNK - 1 else NCH
                    nc.tensor.matmul(ps[:, q + plo:q + phi], a2w,
                                     dr_pair(n0 + plo - 1, 2, phi - plo),
                                     start=False, stop=False, perf_mode=DR)
                    if n == 0:      # l=0 keeps only the right neighbor
                        nc.tensor.matmul(ps[:, q:q + 1], a2s, x8[:, 1:2],
                                         start=False, stop=False)
                    if n == NCHUNK - 1:   # l=L-1 keeps only the left
                        nc.tensor.matmul(ps[:, q + NCH - 1:q + NCH], a2s,
                                         x8[:, L - 2:L - 1],
                                         start=False, stop=False)
                    # A2 @ (x[l-128]+x[l+128]) as one fp8 DoubleRow matmul
                    vlo = 128 if n == 0 else 0
                    vhi = NCH - 128 if n == NCHUNK - 1 else NCH
                    nc.tensor.matmul(ps[:, q + vlo:q + vhi], a2w,
                                     dr_pair(n0 + vlo - 128, 256, vhi - vlo),
                                     start=False, stop=False, perf_mode=DR)
                    if n == 0:      # first image row keeps only down
                        nc.tensor.matmul(ps[:, q:q + 128], a2s,
                                         x8[:, 128:256],
                                         start=False, stop=False)
                    if n == NCHUNK - 1:   # last image row keeps only up
                        nc.tensor.matmul(ps[:, q + NCH - 128:q + NCH], a2s,
                                         x8[:, L - 256:L - 128],
                                         start=False, stop=False)
                    if n == 0:
                        # col-scan wrap: l=j gets x[(h-1)w + j - 1]
                        nc.tensor.matmul(ps[:, q + 1:q + 128], wa2t,
                                         xbf[:, L - Wimg:L - 1],
                                         start=False, stop=False)
                    if n == NCHUNK - 1:
                        # col-scan wrap: l=(h-1)w+j gets x[j+1]
                        nc.tensor.matmul(ps[:, q + NCH - 128:q + NCH - 1],
                                         wa2t, xbf[:, 1:128],
                                         start=False, stop=False)
                # diff branch
                for h in range(2):
                    pso = ps[:, h * NCH:(h + 1) * NCH]
                    o2 = off + h * NCH
                    nc.tensor.matmul(pso, wdt, H[:, o2:o2 + NCH],
                                     start=False, stop=False)
                    nc.tensor.matmul(pso, wdt, V[:, o2:o2 + NCH],
                                     start=False, stop=False)
                # channel contribution (row-tiled, K=32)
                for h in range(2):
                    n = 2 * m + h
                    j = n % 4
                    nc.tensor.matmul(ps[:, h * NCH:h * NCH + NCH],
                                     c2t4[32 * j:32 * j + 32, :],
                                     h1sb[32 * j:32 * j + 32,
                                          (n // 4) * NCH:(n // 4 + 1) * NCH],
                                     start=False, stop=(h == 1),
                                     tile_position=(32 * j, 0))
                dmp = dump.tile([C, NP], bf16, tag="sq")
                if m == NPAIR - 1:
                    # split the last pair's evac/square into halves: the
                    # first half runs while the second half's matmuls finish,
                    # shortening the stats -> AllReduce critical path
                    for h in range(2):
                        q = h * NCH
                        nc.scalar.activation(ypre[:, p0 + q:p0 + q + NCH],
                                             ps[:, q:q + NCH], Act.Identity,
                                             bias=bout[:, 0:1],
                                             accum_out=ysum[:, m + h:m + h + 1])
                        nc.scalar.activation(dmp[:, q:q + NCH],
                                             ps[:, q:q + NCH], Act.Square,
                                             accum_out=ysq[:, m + h:m + h + 1])
                else:
                    nc.scalar.activation(ypre[:, p0:p0 + NP], ps,
                                         Act.Identity, bias=bout[:, 0:1],
                                         accum_out=ysum[:, m:m + 1])
                    nc.scalar.activation(dmp, ps, Act.Square,
                                         accum_out=ysq[:, m:m + 1])

            h1_group(0)
            win_arrays = win_arrays0
            for k in range(NGRP):
                nxt = None
                if k + 1 < NGRP:
                    h1_group(k + 1)
                    nxt = window_arrays(k + 1)
                chunk_pair(2 * k, *win_arrays)
                chunk_pair(2 * k + 1, *win_arrays)
                win_arrays = nxt

            # ---- global BN stats via AllReduce ----
            # ysq tracked z = y - bout (PSUM, pre-bias):
            #   sum(y^2) = sum(z^2) + 2*bout*sum(y) - L*bout^2
            # (2*bout and -L*bout^2 are host-folded into bout2/blsq)
            stats = sm.tile([C, 2], f32)
            nc.vector.tensor_reduce(stats[:, 0:1], ysum, X, Alu.add)
            nc.vector.tensor_reduce(stats[:, 1:2], ysq, X, Alu.add)
            nc.vector.scalar_tensor_tensor(stats[:, 1:2], stats[:, 0:1],
                                           bout2[:, 0:1], stats[:, 1:2],
                                           Alu.mult, Alu.add)
            nc.vector.tensor_tensor(stats[:, 1:2], stats[:, 1:2], blsq,
                                    Alu.add)
            # prefetch the sqrt ACT table while the collective runs
            sqpre = sm.tile([C, 1], f32)
            nc.scalar.activation(sqpre, stats[:, 1:2], Act.Sqrt)
            cc_in = dram.tile([C, 2], f32)
            cc_out = dram.tile([C, 2], f32)
            nc.gpsimd.dma_start(out=cc_in[:], in_=stats)
            nc.gpsimd.collective_compute(
                "AllReduce", Alu.add,
                replica_groups=[list(range(NCORES))],
                ins=[cc_in.opt()], outs=[cc_out.opt()])
            statsr = sm.tile([C, 2], f32)
            nc.gpsimd.dma_start(out=statsr, in_=cc_out[:])

            mean = sm.tile([C, 1], f32)
            ex2 = sm.tile([C, 1], f32)
            nc.vector.tensor_scalar(mean, statsr[:, 0:1], 1.0 / NTOT, None,
                                    Alu.mult)
            nc.vector.tensor_scalar(ex2, statsr[:, 1:2], 1.0 / NTOT, None,
                                    Alu.mult)
            m2 = sm.tile([C, 1], f32)
            nc.vector.tensor_tensor(m2, mean, mean, Alu.mult)
            varep = sm.tile([C, 1], f32)
            nc.vector.tensor_tensor(varep, ex2, m2, Alu.subtract)
            nc.vector.tensor_scalar(varep, varep, EPS_BN, None, Alu.add)
            inv = sm.tile([C, 1], f32)
            nc.vector.reciprocal(inv, varep)
            rstd = sm.tile([C, 1], f32)
            nc.scalar.activation(rstd, inv, Act.Sqrt)
            s_sc = sm.tile([C, 1], f32)
            nc.vector.tensor_tensor(s_sc, rstd, gb[:, 0:1], Alu.mult)
            ms = sm.tile([C, 1], f32)
            nc.vector.tensor_tensor(ms, mean, s_sc, Alu.mult)
            t_sc = sm.tile([C, 1], f32)
            nc.vector.tensor_tensor(t_sc, gb[:, 1:2], ms, Alu.subtract)

            # ---- apply BN (6 windows on DVE, 2 on the slower ACT, in
            #      parallel; output DMAs on the sync + idle gpsimd queues,
            #      keeping the compute queues free) ----
            for i, g in enumerate([0, 2, 3, 4, 6, 7, 1, 5]):
                lo, hi = g * NW, (g + 1) * NW
                ow = owp.tile([C, NW], bf16, tag="ow")
                if i >= 6:
                    nc.scalar.activation(ow, ypre[:, lo:hi], Act.Identity,
                                         bias=t_sc[:, 0:1],
                                         scale=s_sc[:, 0:1])
                else:
                    nc.vector.tensor_scalar(ow, ypre[:, lo:hi],
                                            s_sc[:, 0:1], t_sc[:, 0:1],
                                            Alu.mult, Alu.add)
                # two 1024-col transfers per window, alternating queues,
                # so the post-BN drain is transfer- not issue-bound
                h = (lo + hi) // 2
                e0 = nc.sync if i % 2 == 0 else nc.gpsimd
                e1 = nc.gpsimd if i % 2 == 0 else nc.sync
                e0.dma_start(out=y_ext[:, lo:h], in_=ow[:, 0:NW // 2])
                e1.dma_start(out=y_ext[:, h:hi], in_=ow[:, NW // 2:NW])

    _split_excess_waits(nc)
    return nc


def _fold_weights(inputs):
    f = np.float32
    W_in = inputs["w_spatial_in"].astype(np.float64)
    W_out = inputs["w_spatial_out"].astype(np.float64)
    dw_sp = inputs["w_dw_spatial"][:, 0, :].astype(np.float64)
    W_proj = inputs["w_out_proj"].astype(np.float64)
    W_mlp2 = inputs["w_mlp2"].astype(np.float64)
    dwt = float(inputs["diff_weight"])

    a_sym = dw_sp[:, 0] + dw_sp[:, 2]
    w1 = dw_sp[:, 1]
    A2 = 0.25 * W_proj @ (W_out * a_sym[None, :]) @ W_in
    B3 = W_proj @ (W_out * w1[None, :]) @ W_in + W_proj
    W_d = 0.25 * dwt * W_proj
    C2 = W_proj @ W_mlp2                     # [c, 32]
    bias_out = W_proj @ inputs["b_mlp2"].astype(np.float64)

    bf = ml_dtypes.bfloat16
    f8 = ml_dtypes.float8_e4m3
    a2t8 = (A2.T * ASC).astype(f8)
    wbf = np.concatenate(
        [B3.T.astype(bf), A2.T.astype(bf), W_d.T.astype(bf),
         np.tile(C2.T.astype(bf), (4, 1))], axis=1)
    wf32 = np.concatenate(
        [inputs["w_ch_out"].astype(f), inputs["w_ch_in"].astype(f),
         inputs["w_mlp1"].T.astype(f),
         inputs["w_ch_dw"][:, 0, :].astype(f)], axis=1)
    wsm = np.concatenate(
        [np.tile(inputs["b_mlp1"].astype(f), 4)[:, None],
         bias_out.astype(f)[:, None],
         np.stack([inputs["bn_gamma"], inputs["bn_beta"]], 1).astype(f),
         (2.0 * bias_out).astype(f)[:, None],
         (-float(L) * bias_out ** 2).astype(f)[:, None],
         np.zeros((128, 1), f)], axis=1)
    return {
        "wbf": np.ascontiguousarray(wbf),
        "a2d": np.ascontiguousarray(np.concatenate([a2t8, a2t8], axis=1)),
        "wsm": np.ascontiguousarray(wsm),
        "wf32": np.ascontiguousarray(wf32),
    }


def prepare_in_maps(inputs):
    wmap = _fold_weights(inputs)
    x = np.asarray(inputs["x"]).astype(np.float32)  # [B, C, H, W]
    in_maps = []
    for b in range(NCORES):
        m = dict(wmap)
        xb = x[b].reshape(C, L)
        m["x"] = np.ascontiguousarray(xb.astype(ml_dtypes.bfloat16))
        in_maps.append(m)
    return in_maps


def kernel(**inputs):
    from concourse.bass_utils import run_bass_kernel_spmd

    inputs = {k: np.asarray(v) for k, v in inputs.items()}
    if "nc" not in _CACHE:
        _CACHE["nc"] = _build_program()
    nc = _CACHE["nc"]

    in_maps = prepare_in_maps(inputs)
    res = run_bass_kernel_spmd(nc, in_maps, list(range(NCORES)))
    out = np.stack([np.asarray(res.results[b]["y"]).astype(np.float32)
                    .reshape(C, Himg, Wimg) for b in range(NCORES)])
    return out
